# revision 7
# baseline (speedup 1.0000x reference)
"""Raw-bass pipelined TT-linear kernel (v3).

Math: W (1024x1024) is a rank-20 TT product, so
  y = (x @ Hin) @ [Hout; bias] with Hin (1024,20), Hout (20,1024).
Data-parallel over batch: 8 cores x 2048 rows.

Schedule notes (from NTFF traces of v1/v2):
  - 16 shared SDMA engines give ~420 GB/s/core total, but a single DMA
    channel only sustains ~150-270 GB/s; the queue overlaps ~2 channels
    at a time in issue order. So every stream is split into 0.5 MiB
    half-chunk channels to keep the queue saturated.
  - The gpsimd SWDGE ring has ~4us first-packet latency; weights ride
    the sync HWDGE ring ahead of the inputs instead (hin alone first --
    40 KiB -- so GEMM1 can start as soon as xt0's first half lands).
  - Output channels are gated on input-stream completion: engines
    fair-share across active channels, so an early output channel
    steals bandwidth from the critical input stream 1:1.
  - Outputs go to DRAM in a [chunk, 128, 4096] layout (4 KiB
    descriptors); the host de-transposes.
  - PSUM p1 is zeroed once at the head; GEMM1 uses start=True on each
    column group's first matmul (kc<4), so no per-chunk re-zeroing.
    Rows between the four 20-row group slices stay zero forever, so no
    NaN garbage reaches GEMM2 via t4 (houtb is zero there, but NaN*0
    would still poison it).  t4 rows 96-127 are memset to 1.0 once:
    row 116 is the bias/ones row, rows 96-115 are overwritten by every
    t4copy before GEMM2 reads them, rows 117-127 hit zero houtb rows.

Engine roles:
  sync   : hin, houtb, 8 input half-channels, then out halves H0
  gpsimd : t4 head memsets, out halves H1 (after inputs done), cleanup
  scalar : ACT table warm-up + half of the PSUM->SBUF evacuations
  vector : p1 head memsets, t4 group copies, other half of evacuations
  tensor : matmuls, software-pipelined G1(c+1) before G2(c)
"""

from contextlib import ExitStack

import numpy as np

import concourse.bass as bass
import concourse.mybir as mybir
from concourse.bass_utils import run_bass_kernel_spmd

N_CORES = 8
B_SHARD = 2048
D_IN = 1024
D_OUT = 1024
R = 20
KC = 8
CHUNK = 512
N_CHUNKS = B_SHARD // CHUNK
BT = CHUNK // 128
QPC = 2 * BT  # half-tiles per chunk
P2_BUFS = 6
BIAS_ROW = 116
HIN_COLS = KC * R  # 160
W_COLS = HIN_COLS + D_OUT  # hin ++ houtb

_DT = {"f32": mybir.dt.float32, "bf16": mybir.dt.bfloat16}

# evacuation engine per half-tile (q % 8): v=vector, s=scalar
_EVAC = "vsvsvsvs"


def _eng(q):
    return _EVAC[q % 8]


def _cnt(eng, q):
    """# of halves with index <= q evacuated by `eng`."""
    return sum(1 for i in range(q + 1) if _eng(i) == eng)


def build_nc(compute="bf16", out_bf16=True):
    cdt = _DT[compute]
    odt = mybir.dt.bfloat16 if out_bf16 else mybir.dt.float32
    f32 = mybir.dt.float32

    nc = bass.Bass("TRN2", target_bir_lowering=False, debug=False)

    xt_d = nc.declare_dram_parameter(
        "xt", [N_CHUNKS, 128, KC * CHUNK], cdt, isOutput=False
    )
    wb_d = nc.declare_dram_parameter("wb", [128, W_COLS], cdt, isOutput=False)
    out_d = nc.declare_dram_parameter(
        "out", [N_CHUNKS, 128, BT * D_OUT], odt, isOutput=True
    )

    with ExitStack() as ctx:
        wb_sb = ctx.enter_context(nc.sbuf_tensor("wb_sb", [128, W_COLS], cdt))
        xt_sb = [
            ctx.enter_context(nc.sbuf_tensor(f"xt{i}", [128, KC * CHUNK], cdt))
            for i in range(N_CHUNKS)
        ]
        t4_sb = [
            ctx.enter_context(nc.sbuf_tensor(f"t4{i}", [128, CHUNK], cdt))
            for i in range(2)
        ]
        y_sb = [
            ctx.enter_context(nc.sbuf_tensor(f"y{i}", [128, BT * D_OUT], odt))
            for i in range(N_CHUNKS)
        ]
        p1 = [
            ctx.enter_context(nc.psum_tensor(f"p1{i}", [128, 512], f32))
            for i in range(2)
        ]
        p2 = [
            ctx.enter_context(nc.psum_tensor(f"p2_{i}", [128, 512], f32))
            for i in range(P2_BUFS)
        ]
        # DMA-completion semaphores: a dma_start's then_inc(sem, 16) is 16
        # independent +1s (one per SDMA engine), so only "all 16 done"
        # thresholds are race-free.
        sem_hin = ctx.enter_context(nc.semaphore("sem_hin"))
        sem_hout = ctx.enter_context(nc.semaphore("sem_hout"))
        sem_xh = [
            [ctx.enter_context(nc.semaphore(f"sem_x{c}h{h}")) for h in range(2)]
            for c in range(N_CHUNKS)
        ]
        sem_outc = [
            ctx.enter_context(nc.semaphore(f"sem_outc{i}"))
            for i in range(N_CHUNKS)
        ]
        (sem_mm1, sem_t4, sem_mm2, sem_yv, sem_ys, sem_p1z, sem_ones) = [
            ctx.enter_context(nc.semaphore(n))
            for n in (
                "sem_mm1", "sem_t4", "sem_mm2", "sem_yv", "sem_ys",
                "sem_p1z", "sem_ones",
            )
        ]
        sems = (
            [sem_hin, sem_hout]
            + [s for pair in sem_xh for s in pair]
            + sem_outc
            + [sem_mm1, sem_t4, sem_mm2, sem_yv, sem_ys, sem_p1z, sem_ones]
        )
        nums = sorted(s.num for s in sems)
        assert nums == list(range(nums[0], nums[0] + len(nums))), nums
        sem_range = range(nums[0], nums[-1] + 1)

        sem_of = {"v": sem_yv, "s": sem_ys}

        def evac_wait(engine, q):
            """Wait until evacuation of half-tile q has completed."""
            engine.wait_ge(sem_of[_eng(q)], _cnt(_eng(q), q))

        def half_evac_wait(engine, c, h):
            """Wait until all 4 half-tiles of out half (c, h) are in SBUF."""
            q_last = QPC * c + 4 * h + 3
            for e in "vs":
                engine.wait_ge(sem_of[e], _cnt(e, q_last))

        HC = 4 * CHUNK  # columns per input half-channel

        def in_dma(engine, c, h):
            engine.dma_start(
                out=xt_sb[c][:, h * HC : (h + 1) * HC],
                in_=xt_d[c][:, h * HC : (h + 1) * HC],
            ).then_inc(sem_xh[c][h], 16)

        OC = 2 * D_OUT  # columns per output half-channel

        def out_dma(engine, c, h):
            half_evac_wait(engine, c, h)
            engine.dma_start(
                out=out_d[c][:, h * OC : (h + 1) * OC],
                in_=y_sb[c][:, h * OC : (h + 1) * OC],
            ).then_inc(sem_outc[c], 16)

        with nc.Block() as block:

            @block.sync
            def _(sync):
                # hin leads (tiny): GEMM1(0) needs it + xt0 half 0 only
                sync.dma_start(
                    out=wb_sb[:, 0:HIN_COLS], in_=wb_d[:, 0:HIN_COLS]
                ).then_inc(sem_hin, 16)
                sync.dma_start(
                    out=wb_sb[:, HIN_COLS:], in_=wb_d[:, HIN_COLS:]
                ).then_inc(sem_hout, 16)
                for c in range(N_CHUNKS):
                    for h in range(2):
                        in_dma(sync, c, h)
                # outputs ride behind the input stream in ring order
                for c in range(N_CHUNKS):
                    out_dma(sync, c, 0)

            @block.tensor
            def _(tensor):
                def g1(c):
                    # four column groups concurrent (tile_position=(0,32j));
                    # kc<4 overwrites (start=True), kc>=4 accumulates
                    for kc in range(KC):
                        j = kc % 4
                        if kc == 0:
                            if c == 0:
                                tensor.wait_ge(sem_hin, 16)
                            tensor.wait_ge(sem_xh[c][0], 16)
                            if c < 2:
                                tensor.wait_ge(sem_p1z, c + 1)
                            else:
                                # start=True overwrite must not race
                                # t4copy(c-2)'s read of this bank
                                tensor.wait_ge(sem_t4, c - 1)
                        if kc == 4:
                            tensor.wait_ge(sem_xh[c][1], 16)
                        mm = tensor.matmul(
                            p1[c % 2][32 * j : 32 * j + R, 0:CHUNK],
                            wb_sb[:, kc * R : (kc + 1) * R],
                            xt_sb[c][:, kc * CHUNK : (kc + 1) * CHUNK],
                            start=(kc < 4),
                            stop=(kc == KC - 1),
                            tile_position=(0, 32 * j),
                            skip_group_check=True,
                        )
                        if kc == KC - 1:
                            mm.then_inc(sem_mm1)

                def g2(c):
                    for bt in range(BT):
                        for nh in range(2):
                            q = QPC * c + 2 * bt + nh
                            if q == QPC * c:
                                tensor.wait_ge(sem_t4, c + 1)
                                if c == 0:
                                    tensor.wait_ge(sem_hout, 16)
                                if c < 2:
                                    tensor.wait_ge(sem_ones, c + 1)
                            if q >= P2_BUFS:
                                evac_wait(tensor, q - P2_BUFS)
                            tensor.matmul(
                                p2[q % P2_BUFS][:],
                                t4_sb[c % 2][:, bt * 128 : (bt + 1) * 128],
                                wb_sb[
                                    :,
                                    HIN_COLS + nh * 512 : HIN_COLS + (nh + 1) * 512,
                                ],
                                start=True,
                                stop=True,
                            ).then_inc(sem_mm2)

                for c in range(N_CHUNKS):
                    g1(c)
                    g2(c)

            @block.vector
            def _(vector):
                # one-time p1 zeroing: group-gap rows must stay exactly 0
                # (PSUM garbage could be NaN; NaN*0 poisons GEMM2)
                vector.memset(p1[0][:], 0.0).then_inc(sem_p1z)
                vector.memset(p1[1][:], 0.0).then_inc(sem_p1z)

                def t4copy(c):
                    vector.wait_ge(sem_mm1, c + 1)
                    if c < 2:
                        vector.wait_ge(sem_ones, c + 1)  # head memsets done
                    else:
                        # t4 buffer reuse: all GEMM2 of chunk c-2 done
                        vector.wait_ge(sem_mm2, QPC * (c - 2) + QPC)
                    vector.tensor_copy(
                        t4_sb[c % 2][0:BIAS_ROW, :],
                        p1[c % 2][0:BIAS_ROW, 0:CHUNK],
                    ).then_inc(sem_t4)

                def evacs(c):
                    for bt in range(BT):
                        for nh in range(2):
                            q = QPC * c + 2 * bt + nh
                            if _eng(q) != "v":
                                continue
                            vector.wait_ge(sem_mm2, q + 1)
                            o0 = bt * D_OUT + nh * 512
                            vector.tensor_copy(
                                y_sb[c][:, o0 : o0 + 512],
                                p2[q % P2_BUFS][:],
                            ).then_inc(sem_yv)

                for c in range(N_CHUNKS):
                    t4copy(c)
                    evacs(c)

            @block.scalar
            def _(scalar):
                # dummy copy: pull the one-time ACT_TABLE_LOAD (~1.3us) into
                # the head instead of the first real evacuation
                scalar.wait_ge(sem_ones, 1)
                scalar.copy(y_sb[0][0:1, 0:32], t4_sb[0][0:1, 0:32])
                for c in range(N_CHUNKS):
                    for bt in range(BT):
                        for nh in range(2):
                            q = QPC * c + 2 * bt + nh
                            if _eng(q) != "s":
                                continue
                            scalar.wait_ge(sem_mm2, q + 1)
                            o0 = bt * D_OUT + nh * 512
                            scalar.copy(
                                y_sb[c][:, o0 : o0 + 512],
                                p2[q % P2_BUFS][:],
                            ).then_inc(sem_ys)

            @block.gpsimd
            def _(gpsimd):
                # t4 rows 96-127 <- 1.0 once (partition base must be 32-
                # aligned): row 116 is the bias/ones row; rows 96-115 are
                # re-written by every t4copy before GEMM2 reads them; rows
                # 117-127 hit zero houtb rows (1.0, not garbage, so no NaN).
                for i in range(2):
                    gpsimd.memset(t4_sb[i][96:128, :], 1.0).then_inc(sem_ones)
                # hold H1 outputs until the input stream is fully landed:
                # engines fair-share across active channels, so an early
                # output channel would slow the critical input stream.
                gpsimd.wait_ge(sem_xh[N_CHUNKS - 1][1], 16)
                for c in range(N_CHUNKS):
                    out_dma(gpsimd, c, 1)
                for c in range(N_CHUNKS):
                    gpsimd.wait_ge(sem_outc[c], 32)
                # leave semaphores clean for any re-execution
                gpsimd.dma_reset(sem_range)
                gpsimd.sem_clear(sem_range)

    return nc


def host_prep(x, cores, bias, np_dt):
    A = cores[0][0].astype(np.float64)
    for G in cores[1:4]:
        G = G.astype(np.float64)
        A = np.einsum("ir,rjs->ijs", A, G).reshape(-1, G.shape[2])
    H = cores[4].astype(np.float64)
    for G in cores[5:]:
        G = G.astype(np.float64)
        H = np.einsum("pNq,qnr->pNnr", H, G).reshape(H.shape[0], -1, G.shape[2])
    H = H.reshape(H.shape[0], -1)  # (20, 1024)

    hin = np.ascontiguousarray(
        A.reshape(KC, 128, R).transpose(1, 0, 2).reshape(128, KC * R)
    )
    # Hout replicated into the four 32-row column groups + bias in row 116;
    # rows outside the rank blocks stay exactly 0 (t4 garbage protection)
    houtb = np.zeros((128, D_OUT), dtype=np.float64)
    for j in range(4):
        houtb[32 * j : 32 * j + R, :] = H
    houtb[BIAS_ROW, :] = bias.astype(np.float64)
    wb = np.concatenate([hin, houtb], axis=1).astype(np_dt)  # [128, 1184]
    xt = np.ascontiguousarray(
        x.reshape(N_CORES, N_CHUNKS, CHUNK, KC, 128).transpose(0, 1, 4, 3, 2)
    ).astype(np_dt).reshape(N_CORES, N_CHUNKS, 128, KC * CHUNK)
    return xt, wb


def unshard_out(raw):
    """[N_CHUNKS, 128, BT*D_OUT] -> [B_SHARD, D_OUT]"""
    return (
        raw.reshape(N_CHUNKS, 128, BT, D_OUT)
        .transpose(0, 2, 1, 3)
        .reshape(B_SHARD, D_OUT)
    )


_NC_CACHE = {}


def run(x, cores, bias, compute="bf16", out_bf16=True, trace=False):
    np_dt = np.dtype(mybir.dt.np(_DT[compute]))
    xt, wb = host_prep(x, cores, bias, np_dt)
    key = (compute, out_bf16)
    if key not in _NC_CACHE:
        _NC_CACHE[key] = build_nc(compute, out_bf16)
    nc = _NC_CACHE[key]
    in_maps = [{"xt": xt[i], "wb": wb} for i in range(N_CORES)]
    res = run_bass_kernel_spmd(nc, in_maps, list(range(N_CORES)), trace=trace)
    out = np.concatenate(
        [unshard_out(res.results[i]["out"]) for i in range(N_CORES)], axis=0
    )
    return out.astype(np.float32), res


def kernel(x, core0, core1, core2, core3, core4, core5, core6, core7, bias):
    cores = (core0, core1, core2, core3, core4, core5, core6, core7)
    out, _ = run(
        np.asarray(x, dtype=np.float32),
        [np.asarray(c, dtype=np.float32) for c in cores],
        np.asarray(bias, dtype=np.float32),
    )
    return out
